# revision 23
# baseline (speedup 1.0000x reference)
"""FASTMultiHeadAttention v2 (polynomial softmax + RPE bias, causal) on 8 trn2 cores.

Math per (b,h):   s[i,j] = q_i.k_j + q_i.rpe[n-1+i-j]
                  score  = 1 + s + 0.5 s^2    (= 0.5[(s+1)^2 + 1], 0.5 cancels)
                  o_i    = sum_{j<=i} score v_j / sum_{j<=i} score

Device pipeline per (b,h) [B*H = 32 units, 4 per core], all fp16 I/O:
  - m2r[ii,t]  = q_i . rpeR-band           (PE h64 row-packed, psum f32)
  - m2r evac   -> SBUF fp16                (ACT/DVE alternating)
  - bias tile  = shear-read of m2r         (SBUF->SBUF DMA, per row-tile)
  - psum_s     = qT.T @ kT  (PE h0)  then  += idn.T @ bias   (PE "bias-MM":
                 the bias add runs on the PE as an identity-weights matmul,
                 so the evac is single-source and splits across ACT+DVE)
  - w = psum_s + 1 evac     -> SBUF fp16   (ACT Copy bias=1 / DVE ts_add 1)
  - diag mask via gpsimd.affine_select (also kills sheared garbage tail)
  - wT blocks via PE transpose -> psum fp16 (16-bit psum => 2x-speed square)
  - scoreT     = psum_t^2 -> SBUF fp16     (DVE tensor_mul / ACT Square)
  - o accum    : psum_o[i,d] += scoreT-block.T @ va-block  (scoreT as
                 WEIGHTS, va as stream => output lands [i,d] directly,
                 no final transposes; col 64 of va = ones => denominator)
  - psum_o -> SBUF -> DRAM raw; the cumsum(+1-term) correction and the
    divide run on the host.

4-stage software pipeline over the 32 (bh, row-tile) units so the PE never
waits on ACT/DVE/DMA turnaround: iter n runs m2r(n) | s+bias(n-1) |
transpose(n-2) | o-accum(n-3).
"""

import sys

if "/opt/trn_rl_repo" not in sys.path:
    sys.path.insert(0, "/opt/trn_rl_repo")

import ml_dtypes  # noqa: F401
import numpy as np

import bass_rust
import concourse.bacc as bacc
import concourse.bass as bass  # noqa: F401
import concourse.mybir as mybir
import concourse.tile as tile
from concourse.bass_utils import run_bass_kernel_spmd

F32 = mybir.dt.float32
F16 = mybir.dt.float16

B, H, N, D = 2, 16, 1024, 64
NBH = B * H  # 32
N_CORES = 8
BH_PER_CORE = NBH // N_CORES  # 4
NT = N // 128  # 8 row tiles
MROW = 1152  # m2r row-buffer width (1024 band + 128 garbage slack)


def _chunks(total):
    out = []
    c = 0
    while c < total:
        out.append((c, min(512, total - c)))
        c += 512
    return out


CHUNKS = {I: _chunks(128 * (I + 1)) for I in range(NT)}


def _shear_ap(t_ap, row_elems, offset, width):
    """AP reading t[p, offset - p + m] for m in [0, width)."""
    cp = t_ap.copy()
    cp.ap = bass_rust.VecI64Pair([[row_elems - 1, 128], [1, width]])
    cp.offset = offset
    return cp


def _skew_ap(t_ap, row_elems, offset, width):
    """AP addressing t[p, offset + p + m] for m in [0, width)."""
    cp = t_ap.copy()
    cp.ap = bass_rust.VecI64Pair([[row_elems + 1, 128], [1, width]])
    cp.offset = offset
    return cp


def build_program():
    nc = bacc.Bacc(
        "TRN2", target_bir_lowering=False, debug=False, num_devices=N_CORES
    )

    qT_d = nc.dram_tensor("qT", [BH_PER_CORE, 64, N], F16, kind="ExternalInput").ap()
    kT_d = nc.dram_tensor("kT", [BH_PER_CORE, 64, N], F16, kind="ExternalInput").ap()
    va_d = nc.dram_tensor("va", [BH_PER_CORE, N, 65], F16, kind="ExternalInput").ap()
    rpe_d = nc.dram_tensor("rpeR", [64, 1024], F16, kind="ExternalInput").ap()
    idn_d = nc.dram_tensor("idn", [128, 128], F16, kind="ExternalInput").ap()
    o_d = nc.dram_tensor(
        "o", [BH_PER_CORE, NT, 128, 65], F32, kind="ExternalOutput"
    ).ap()

    with tile.TileContext(nc) as tc:
        with (
            tc.tile_pool(name="const", bufs=1) as cpool,
            tc.tile_pool(name="io", bufs=2) as io,
            tc.tile_pool(name="m2r", bufs=3) as m2rp,
            tc.tile_pool(name="bias", bufs=4) as bp,
            tc.tile_pool(name="wrow", bufs=3) as wp,
            tc.tile_pool(name="sct", bufs=3) as scp,
            tc.tile_pool(name="srow", bufs=3) as srp,
            tc.tile_pool(name="fin", bufs=2) as fp,
            tc.tile_pool(name="psm", bufs=2, space="PSUM") as ps_m,
            tc.tile_pool(name="pss", bufs=2, space="PSUM") as ps_s,
            tc.tile_pool(name="pst", bufs=2, space="PSUM") as ps_t,
            tc.tile_pool(name="pso", bufs=2, space="PSUM") as ps_o,
        ):
            idn = cpool.tile([128, 128], F16)
            nc.sync.dma_start(idn[:], idn_d[:])
            # rpeR only feeds the h64 row-packed m2r matmuls
            rpeR = cpool.tile([128, 1024], F16)
            nc.sync.dma_start(rpeR[64:128, :], rpe_d[:])

            def load_io(m):
                qT = io.tile([128, N], F16, tag="qT")
                nc.gpsimd.dma_start(qT[64:128, :], qT_d[m])
                nc.gpsimd.dma_start(qT[0:64, :], qT_d[m])
                kT = io.tile([64, N], F16, tag="kT")
                nc.gpsimd.dma_start(kT[:], kT_d[m])
                va = io.tile([128, NT * 65], F16, tag="va")
                nc.gpsimd.dma_start(
                    va[:].rearrange("p (a d) -> p a d", a=NT),
                    va_d[m].rearrange("(a b) d -> b a d", a=NT),
                )
                return (qT, kT, va)

            TOT = BH_PER_CORE * NT  # 32 pipeline units
            state = {}
            cur_io = None
            cur_out = None
            cur_po = None
            ctr = {"m2r": 0, "w": 0, "sq": 0, "oe": 0}

            for it in range(TOT + 4):
                # ---- stages A+B interleaved: m2r(it) on PE rows 64-127
                # while s(it-2) runs on rows 0-63 (concurrent row groups) ----
                ub = it - 2
                stA = stB = None
                if it < TOT:
                    m, I = divmod(it, NT)
                    stA = state[it] = {}
                    if I == 0:
                        cur_io = load_io(0) if m == 0 else next_io
                        cur_out = fp.tile([128, NT * 65], F32, tag="ofin")
                    if I == 4 and m + 1 < BH_PER_CORE:
                        next_io = load_io(m + 1)
                    stA["io"] = cur_io
                    stA["ofin"] = cur_out
                    IA = I
                    qTA = cur_io[0]
                    widthA = 128 * (IA + 1)
                    u0 = 896 - 128 * IA
                    m2r = m2rp.tile([128, MROW], F16, tag="m2r")
                    nc.gpsimd.memset(m2r[:, widthA : widthA + 127], 0.0)
                if 0 <= ub < TOT:
                    mB, IB = divmod(ub, NT)
                    stB = state[ub]
                    qTB, kTB, vaB = stB["io"]
                    biasB = stB["bias"]
                    wrow = wp.tile([128, 1024], F16, tag="wrow")

                if stA is not None:
                    for c, wd in CHUNKS[IA]:
                        pm = ps_m.tile([128, 512], F32, tag="m")
                        nc.tensor.matmul(
                            pm[:, :wd],
                            qTA[64:128, 128 * IA : 128 * (IA + 1)],
                            rpeR[64:128, u0 + c : u0 + c + wd],
                            start=True,
                            stop=True,
                            tile_position=(64, 0),
                        )
                        if ctr["m2r"] % 3 == 0:
                            nc.scalar.copy(m2r[:, c : c + wd], pm[:, :wd])
                        else:
                            nc.vector.tensor_copy(m2r[:, c : c + wd], pm[:, :wd])
                        ctr["m2r"] += 1
                    bias = bp.tile([128, 1024], F16, tag="bias")
                    sh_eng = nc.sync
                    sh_eng.dma_start(
                        bias[:, :widthA], _shear_ap(m2r[:], MROW, 127, widthA)
                    )
                    stA["bias"] = bias
                if stB is not None:
                    for c, wd in CHUNKS[IB]:
                        use_stt = ctr["w"] % 2 == 1
                        ps = ps_s.tile([128, 512], F32, tag="s")
                        nc.tensor.matmul(
                            ps[:, :wd],
                            qTB[0:64, 128 * IB : 128 * (IB + 1)],
                            kTB[:, c : c + wd],
                            start=True,
                            stop=use_stt,
                            tile_position=(0, 0),
                        )
                        if use_stt:
                            # bias add + (+1) on the DVE, straight to fp16
                            nc.vector.scalar_tensor_tensor(
                                wrow[:, c : c + wd],
                                ps[:, :wd],
                                1.0,
                                biasB[:, c : c + wd],
                                mybir.AluOpType.add,
                                mybir.AluOpType.add,
                            )
                        else:
                            # bias add on the PE (identity-weights matmul),
                            # then a single-source +1 evac on ACT
                            nc.tensor.matmul(
                                ps[:, :wd],
                                idn[:],
                                biasB[:, c : c + wd],
                                start=False,
                                stop=True,
                            )
                            nc.scalar.activation(
                                wrow[:, c : c + wd],
                                ps[:, :wd],
                                mybir.ActivationFunctionType.Copy,
                                bias=1.0,
                            )
                        ctr["w"] += 1
                    # causal mask on the diagonal block: keep jj <= ii
                    # (also wipes the sheared-garbage upper triangle)
                    nc.gpsimd.affine_select(
                        wrow[:, 128 * IB : 128 * (IB + 1)],
                        wrow[:, 128 * IB : 128 * (IB + 1)],
                        pattern=[[-1, 128]],
                        compare_op=mybir.AluOpType.is_ge,
                        fill=0.0,
                        base=0,
                        channel_multiplier=1,
                    )
                    # square in SBUF (2-byte operands -> DVE 2x mode); squaring
                    # commutes with the transpose, so the post-transpose evac
                    # becomes a plain copy. The diag block squares after the
                    # mask; everything earlier squares as soon as it exists.
                    srow = srp.tile([128, 1024], F16, tag="srow")
                    widthB = 128 * (IB + 1)
                    if widthB > 128:
                        nc.vector.tensor_mul(
                            srow[:, : widthB - 128],
                            wrow[:, : widthB - 128],
                            wrow[:, : widthB - 128],
                        )
                    nc.vector.tensor_mul(
                        srow[:, widthB - 128 : widthB],
                        wrow[:, widthB - 128 : widthB],
                        wrow[:, widthB - 128 : widthB],
                    )
                    stB["srow"] = srow

                # ---- stage C: transposes + square (unit it-3) ----
                uc = it - 3
                if 0 <= uc < TOT:
                    m, I = divmod(uc, NT)
                    st = state[uc]
                    srow = st["srow"]
                    scoreT = scp.tile([128, 1024], F16, tag="sc")
                    for c, wd in CHUNKS[I]:
                        pt_ = ps_t.tile([128, 512], F32, tag="t")
                        for b in range(0, wd, 128):
                            # transpose score-block as a regular matmul:
                            # out = s.T @ idn runs at full clock (FWL)
                            nc.tensor.matmul(
                                pt_[:, b : b + 128],
                                srow[:, c + b : c + b + 128],
                                idn[:],
                                start=True,
                                stop=True,
                            )
                        if ctr["sq"] % 2 == 0:
                            nc.scalar.copy(scoreT[:, c : c + wd], pt_[:, :wd])
                        else:
                            nc.vector.tensor_copy(scoreT[:, c : c + wd], pt_[:, :wd])
                        ctr["sq"] += 1
                    st["scoreT"] = scoreT

                # ---- stage D: output accumulation (unit it-4) ----
                ud = it - 4
                if 0 <= ud < TOT:
                    m, I = divmod(ud, NT)
                    st = state[ud]
                    scoreT = st["scoreT"]
                    va = st["io"][2]
                    ofin = st["ofin"]
                    slot = I % 4
                    if slot == 0:
                        cur_po = ps_o.tile([128, 260], F32, tag="o")
                    po = cur_po
                    for J in range(I + 1):
                        nc.tensor.matmul(
                            po[:, 65 * slot : 65 * slot + 65],
                            scoreT[:, 128 * J : 128 * (J + 1)],
                            va[:, 65 * J : 65 * (J + 1)],
                            start=(J == 0),
                            stop=(J == I),
                        )
                    if slot == 3:
                        g = I // 4
                        if ctr["oe"] % 2 == 0:
                            nc.scalar.copy(ofin[:, 260 * g : 260 * (g + 1)], po[:])
                        else:
                            nc.vector.tensor_copy(
                                ofin[:, 260 * g : 260 * (g + 1)], po[:]
                            )
                        ctr["oe"] += 1
                    if I == NT - 1:
                        nc.scalar.dma_start(
                            o_d[m].rearrange("a b d -> b a d"),
                            ofin[:].rearrange("p (a d) -> p a d", a=NT),
                        )
                    del state[ud]

    nc.compile()
    return nc


_NC_CACHE = {}


def get_program():
    if "nc" not in _NC_CACHE:
        _NC_CACHE["nc"] = build_program()
    return _NC_CACHE["nc"]


def prepare_inputs(q, k, v, rpe_matrix):
    """Host-side prep: returns per-core input maps (all fp16)."""
    q = np.asarray(q, dtype=np.float32).reshape(NBH, N, D)
    k = np.asarray(k, dtype=np.float32).reshape(NBH, N, D)
    v = np.asarray(v, dtype=np.float32).reshape(NBH, N, D)
    rpe = np.asarray(rpe_matrix, dtype=np.float32)

    qT = np.ascontiguousarray(q.transpose(0, 2, 1)).astype(np.float16)
    kT = np.ascontiguousarray(k.transpose(0, 2, 1)).astype(np.float16)
    va = np.concatenate([v, np.ones((NBH, N, 1), np.float32)], axis=2).astype(
        np.float16
    )  # [32, 1024, 65]

    # reversed rpe band: rpeR[:, u] = rpe[2046 - u] for u in [0, 1024)
    rpeR = np.ascontiguousarray(rpe[2046:1022:-1].T).astype(np.float16)  # [64, 1024]
    idn = np.eye(128, dtype=np.float16)

    in_maps = []
    for c in range(N_CORES):
        sl = slice(c * BH_PER_CORE, (c + 1) * BH_PER_CORE)
        in_maps.append(
            {
                "qT": np.ascontiguousarray(qT[sl]),
                "kT": np.ascontiguousarray(kT[sl]),
                "va": np.ascontiguousarray(va[sl]),
                "rpeR": rpeR,
                "idn": idn,
            }
        )
    return in_maps


def run(q, k, v, rpe_matrix, trace=False):
    nc = get_program()
    in_maps = prepare_inputs(q, k, v, rpe_matrix)
    res = run_bass_kernel_spmd(nc, in_maps, list(range(N_CORES)), trace=trace)
    dev = np.stack(
        [np.asarray(res.results[c]["o"]) for c in range(N_CORES)]
    )  # [8, 4, 8, 128, 65]
    dev = dev.reshape(NBH, N, 65).astype(np.float64)

    # host-side "+1"-term correction (cumsum of [v, ones]) and the divide
    v64 = np.asarray(v, dtype=np.float64).reshape(NBH, N, D)
    va64 = np.concatenate([v64, np.ones((NBH, N, 1), np.float64)], axis=2)
    pt = np.cumsum(va64, axis=1)  # [32, 1024, 65]
    num = dev[..., :64] + pt[..., :64]
    den = dev[..., 64:65] + pt[..., 64:65]
    o = (num / den).astype(np.float32).reshape(B, H, N, D)
    return o, res


def kernel(q, k, v, drop_noise=None, rpe_matrix=None, p=2, **kw):
    o, _ = run(q, k, v, rpe_matrix)
    return o


if __name__ == "__main__":
    rng = np.random.default_rng(0)
    q = rng.standard_normal((B, H, N, D), dtype=np.float32)
    k = rng.standard_normal((B, H, N, D), dtype=np.float32)
    v = rng.standard_normal((B, H, N, D), dtype=np.float32)
    rpe = rng.standard_normal((2 * N - 1, D), dtype=np.float32)
    o, _ = run(q, k, v, rpe)
    print("out", o.shape, o.dtype, np.abs(o).max())


# revision 24
# speedup vs baseline: 1.0413x; 1.0413x over previous
"""FASTMultiHeadAttention v2 (polynomial softmax + RPE bias, causal) on 8 trn2 cores.

Math per (b,h):   s[i,j] = q_i.k_j + q_i.rpe[n-1+i-j]
                  score  = 1 + s + 0.5 s^2    (= 0.5[(s+1)^2 + 1], 0.5 cancels)
                  o_i    = sum_{j<=i} score v_j / sum_{j<=i} score

Device pipeline per (b,h) [B*H = 32 units, 4 per core], all fp16 I/O:
  - m2r[ii,t]  = q_i . rpeR-band           (PE h64 row-packed, psum f32)
  - m2r evac   -> SBUF fp16                (ACT/DVE alternating)
  - bias tile  = shear-read of m2r         (SBUF->SBUF DMA, per row-tile)
  - psum_s     = qT.T @ kT  (PE h0)  then  += idn.T @ bias   (PE "bias-MM":
                 the bias add runs on the PE as an identity-weights matmul,
                 so the evac is single-source and splits across ACT+DVE)
  - w = psum_s + 1 evac     -> SBUF fp16   (ACT Copy bias=1 / DVE ts_add 1)
  - diag mask via gpsimd.affine_select (also kills sheared garbage tail)
  - wT blocks via PE transpose -> psum fp16 (16-bit psum => 2x-speed square)
  - scoreT     = psum_t^2 -> SBUF fp16     (DVE tensor_mul / ACT Square)
  - o accum    : psum_o[i,d] += scoreT-block.T @ va-block  (scoreT as
                 WEIGHTS, va as stream => output lands [i,d] directly,
                 no final transposes; col 64 of va = ones => denominator)
  - psum_o -> SBUF -> DRAM raw; the cumsum(+1-term) correction and the
    divide run on the host.

4-stage software pipeline over the 32 (bh, row-tile) units so the PE never
waits on ACT/DVE/DMA turnaround: iter n runs m2r(n) | s+bias(n-1) |
transpose(n-2) | o-accum(n-3).
"""

import sys

if "/opt/trn_rl_repo" not in sys.path:
    sys.path.insert(0, "/opt/trn_rl_repo")

import ml_dtypes  # noqa: F401
import numpy as np

import bass_rust
import concourse.bacc as bacc
import concourse.bass as bass  # noqa: F401
import concourse.mybir as mybir
import concourse.tile as tile
from concourse.bass_utils import run_bass_kernel_spmd

F32 = mybir.dt.float32
F16 = mybir.dt.float16

B, H, N, D = 2, 16, 1024, 64
NBH = B * H  # 32
N_CORES = 8
BH_PER_CORE = NBH // N_CORES  # 4
NT = N // 128  # 8 row tiles
MROW = 1152  # m2r row-buffer width (1024 band + 128 garbage slack)


def _chunks(total):
    out = []
    c = 0
    while c < total:
        out.append((c, min(512, total - c)))
        c += 512
    return out


CHUNKS = {I: _chunks(128 * (I + 1)) for I in range(NT)}


def _shear_ap(t_ap, row_elems, offset, width):
    """AP reading t[p, offset - p + m] for m in [0, width)."""
    cp = t_ap.copy()
    cp.ap = bass_rust.VecI64Pair([[row_elems - 1, 128], [1, width]])
    cp.offset = offset
    return cp


def _skew_ap(t_ap, row_elems, offset, width):
    """AP addressing t[p, offset + p + m] for m in [0, width)."""
    cp = t_ap.copy()
    cp.ap = bass_rust.VecI64Pair([[row_elems + 1, 128], [1, width]])
    cp.offset = offset
    return cp


def build_program():
    nc = bacc.Bacc(
        "TRN2", target_bir_lowering=False, debug=False, num_devices=N_CORES
    )

    qT_d = nc.dram_tensor("qT", [BH_PER_CORE, 64, N], F16, kind="ExternalInput").ap()
    kT_d = nc.dram_tensor("kT", [BH_PER_CORE, 64, N], F16, kind="ExternalInput").ap()
    va_d = nc.dram_tensor("va", [BH_PER_CORE, N, 65], F16, kind="ExternalInput").ap()
    rpe_d = nc.dram_tensor("rpeR", [64, 1024], F16, kind="ExternalInput").ap()
    idn_d = nc.dram_tensor("idn", [128, 128], F16, kind="ExternalInput").ap()
    o_d = nc.dram_tensor(
        "o", [BH_PER_CORE, NT, 128, 65], F32, kind="ExternalOutput"
    ).ap()

    with tile.TileContext(nc) as tc:
        with (
            tc.tile_pool(name="const", bufs=1) as cpool,
            tc.tile_pool(name="io", bufs=2) as io,
            tc.tile_pool(name="m2r", bufs=3) as m2rp,
            tc.tile_pool(name="bias", bufs=4) as bp,
            tc.tile_pool(name="wrow", bufs=3) as wp,
            tc.tile_pool(name="sct", bufs=3) as scp,
            tc.tile_pool(name="srow", bufs=3) as srp,
            tc.tile_pool(name="fin", bufs=2) as fp,
            tc.tile_pool(name="psm", bufs=2, space="PSUM") as ps_m,
            tc.tile_pool(name="pss", bufs=2, space="PSUM") as ps_s,
            tc.tile_pool(name="pst", bufs=2, space="PSUM") as ps_t,
            tc.tile_pool(name="pso", bufs=2, space="PSUM") as ps_o,
        ):
            idn = cpool.tile([128, 128], F16)
            nc.sync.dma_start(idn[:], idn_d[:])
            # rpeR only feeds the h64 row-packed m2r matmuls
            rpeR = cpool.tile([128, 1024], F16)
            nc.sync.dma_start(rpeR[64:128, :], rpe_d[:])

            def load_io(m, eng=None):
                eng = eng or nc.gpsimd
                qT = io.tile([128, N], F16, tag="qT")
                eng.dma_start(qT[64:128, :], qT_d[m])
                eng.dma_start(qT[0:64, :], qT_d[m])
                kT = io.tile([64, N], F16, tag="kT")
                eng.dma_start(kT[:], kT_d[m])
                va = io.tile([128, NT * 65], F16, tag="va")
                eng.dma_start(
                    va[:].rearrange("p (a d) -> p a d", a=NT),
                    va_d[m].rearrange("(a b) d -> b a d", a=NT),
                )
                return (qT, kT, va)

            TOT = BH_PER_CORE * NT  # 32 pipeline units
            state = {}
            cur_io = None
            cur_out = None
            cur_po = None
            ctr = {"m2r": 0, "w": 0, "sq": 0, "oe": 0}

            for it in range(TOT + 4):
                # ---- stages A+B interleaved: m2r(it) on PE rows 64-127
                # while s(it-2) runs on rows 0-63 (concurrent row groups) ----
                ub = it - 2
                stA = stB = None
                if it < TOT:
                    m, I = divmod(it, NT)
                    stA = state[it] = {}
                    if I == 0:
                        cur_io = load_io(0, nc.sync) if m == 0 else next_io
                        cur_out = fp.tile([128, NT * 65], F32, tag="ofin")
                    if I == 4 and m + 1 < BH_PER_CORE:
                        next_io = load_io(m + 1)
                    stA["io"] = cur_io
                    stA["ofin"] = cur_out
                    IA = I
                    qTA = cur_io[0]
                    widthA = 128 * (IA + 1)
                    u0 = 896 - 128 * IA
                    m2r = m2rp.tile([128, MROW], F16, tag="m2r")
                    nc.gpsimd.memset(m2r[:, widthA : widthA + 127], 0.0)
                if 0 <= ub < TOT:
                    mB, IB = divmod(ub, NT)
                    stB = state[ub]
                    qTB, kTB, vaB = stB["io"]
                    biasB = stB["bias"]
                    wrow = wp.tile([128, 1024], F16, tag="wrow")

                if stA is not None:
                    for c, wd in CHUNKS[IA]:
                        pm = ps_m.tile([128, 512], F32, tag="m")
                        nc.tensor.matmul(
                            pm[:, :wd],
                            qTA[64:128, 128 * IA : 128 * (IA + 1)],
                            rpeR[64:128, u0 + c : u0 + c + wd],
                            start=True,
                            stop=True,
                            tile_position=(64, 0),
                        )
                        if ctr["m2r"] % 3 == 0:
                            nc.scalar.copy(m2r[:, c : c + wd], pm[:, :wd])
                        else:
                            nc.vector.tensor_copy(m2r[:, c : c + wd], pm[:, :wd])
                        ctr["m2r"] += 1
                    bias = bp.tile([128, 1024], F16, tag="bias")
                    sh_eng = nc.sync
                    sh_eng.dma_start(
                        bias[:, :widthA], _shear_ap(m2r[:], MROW, 127, widthA)
                    )
                    stA["bias"] = bias
                if stB is not None:
                    for c, wd in CHUNKS[IB]:
                        use_stt = ctr["w"] % 3 != 0
                        ps = ps_s.tile([128, 512], F32, tag="s")
                        nc.tensor.matmul(
                            ps[:, :wd],
                            qTB[0:64, 128 * IB : 128 * (IB + 1)],
                            kTB[:, c : c + wd],
                            start=True,
                            stop=use_stt,
                            tile_position=(0, 0),
                        )
                        if use_stt:
                            # bias add + (+1) on the DVE, straight to fp16
                            nc.vector.scalar_tensor_tensor(
                                wrow[:, c : c + wd],
                                ps[:, :wd],
                                1.0,
                                biasB[:, c : c + wd],
                                mybir.AluOpType.add,
                                mybir.AluOpType.add,
                            )
                        else:
                            # bias add on the PE (identity-weights matmul),
                            # then a single-source +1 evac on ACT
                            nc.tensor.matmul(
                                ps[:, :wd],
                                idn[:],
                                biasB[:, c : c + wd],
                                start=False,
                                stop=True,
                            )
                            nc.scalar.activation(
                                wrow[:, c : c + wd],
                                ps[:, :wd],
                                mybir.ActivationFunctionType.Copy,
                                bias=1.0,
                            )
                        ctr["w"] += 1
                    # causal mask on the diagonal block: keep jj <= ii
                    # (also wipes the sheared-garbage upper triangle)
                    nc.gpsimd.affine_select(
                        wrow[:, 128 * IB : 128 * (IB + 1)],
                        wrow[:, 128 * IB : 128 * (IB + 1)],
                        pattern=[[-1, 128]],
                        compare_op=mybir.AluOpType.is_ge,
                        fill=0.0,
                        base=0,
                        channel_multiplier=1,
                    )
                    # square in SBUF (2-byte operands -> DVE 2x mode); squaring
                    # commutes with the transpose, so the post-transpose evac
                    # becomes a plain copy. The diag block squares after the
                    # mask; everything earlier squares as soon as it exists.
                    srow = srp.tile([128, 1024], F16, tag="srow")
                    widthB = 128 * (IB + 1)
                    sq_eng = nc.vector if ctr["sq"] % 2 == 0 else nc.gpsimd
                    ctr["sq"] += 1
                    if widthB > 128:
                        sq_eng.tensor_mul(
                            srow[:, : widthB - 128],
                            wrow[:, : widthB - 128],
                            wrow[:, : widthB - 128],
                        )
                    sq_eng.tensor_mul(
                        srow[:, widthB - 128 : widthB],
                        wrow[:, widthB - 128 : widthB],
                        wrow[:, widthB - 128 : widthB],
                    )
                    stB["srow"] = srow

                # ---- stage C: transposes + square (unit it-3) ----
                uc = it - 3
                if 0 <= uc < TOT:
                    m, I = divmod(uc, NT)
                    st = state[uc]
                    srow = st["srow"]
                    scoreT = scp.tile([128, 1024], F16, tag="sc")
                    for c, wd in CHUNKS[I]:
                        pt_ = ps_t.tile([128, 512], F32, tag="t")
                        for b in range(0, wd, 128):
                            # transpose score-block as a regular matmul:
                            # out = s.T @ idn runs at full clock (FWL)
                            nc.tensor.matmul(
                                pt_[:, b : b + 128],
                                srow[:, c + b : c + b + 128],
                                idn[:],
                                start=True,
                                stop=True,
                            )
                        if ctr["sq"] % 2 == 0:
                            nc.scalar.copy(scoreT[:, c : c + wd], pt_[:, :wd])
                        else:
                            nc.vector.tensor_copy(scoreT[:, c : c + wd], pt_[:, :wd])
                        ctr["sq"] += 1
                    st["scoreT"] = scoreT

                # ---- stage D: output accumulation (unit it-4) ----
                ud = it - 4
                if 0 <= ud < TOT:
                    m, I = divmod(ud, NT)
                    st = state[ud]
                    scoreT = st["scoreT"]
                    va = st["io"][2]
                    ofin = st["ofin"]
                    slot = I % 4
                    if slot == 0:
                        cur_po = ps_o.tile([128, 260], F32, tag="o")
                    po = cur_po
                    for J in range(I + 1):
                        nc.tensor.matmul(
                            po[:, 65 * slot : 65 * slot + 65],
                            scoreT[:, 128 * J : 128 * (J + 1)],
                            va[:, 65 * J : 65 * (J + 1)],
                            start=(J == 0),
                            stop=(J == I),
                        )
                    if slot == 3:
                        g = I // 4
                        if ctr["oe"] % 2 == 0:
                            nc.scalar.copy(ofin[:, 260 * g : 260 * (g + 1)], po[:])
                        else:
                            nc.vector.tensor_copy(
                                ofin[:, 260 * g : 260 * (g + 1)], po[:]
                            )
                        ctr["oe"] += 1
                    if I == NT - 1:
                        nc.scalar.dma_start(
                            o_d[m].rearrange("a b d -> b a d"),
                            ofin[:].rearrange("p (a d) -> p a d", a=NT),
                        )
                    del state[ud]

    nc.compile()
    return nc


_NC_CACHE = {}


def get_program():
    if "nc" not in _NC_CACHE:
        _NC_CACHE["nc"] = build_program()
    return _NC_CACHE["nc"]


def prepare_inputs(q, k, v, rpe_matrix):
    """Host-side prep: returns per-core input maps (all fp16)."""
    q = np.asarray(q, dtype=np.float32).reshape(NBH, N, D)
    k = np.asarray(k, dtype=np.float32).reshape(NBH, N, D)
    v = np.asarray(v, dtype=np.float32).reshape(NBH, N, D)
    rpe = np.asarray(rpe_matrix, dtype=np.float32)

    qT = np.ascontiguousarray(q.transpose(0, 2, 1)).astype(np.float16)
    kT = np.ascontiguousarray(k.transpose(0, 2, 1)).astype(np.float16)
    va = np.concatenate([v, np.ones((NBH, N, 1), np.float32)], axis=2).astype(
        np.float16
    )  # [32, 1024, 65]

    # reversed rpe band: rpeR[:, u] = rpe[2046 - u] for u in [0, 1024)
    rpeR = np.ascontiguousarray(rpe[2046:1022:-1].T).astype(np.float16)  # [64, 1024]
    idn = np.eye(128, dtype=np.float16)

    in_maps = []
    for c in range(N_CORES):
        sl = slice(c * BH_PER_CORE, (c + 1) * BH_PER_CORE)
        in_maps.append(
            {
                "qT": np.ascontiguousarray(qT[sl]),
                "kT": np.ascontiguousarray(kT[sl]),
                "va": np.ascontiguousarray(va[sl]),
                "rpeR": rpeR,
                "idn": idn,
            }
        )
    return in_maps


def run(q, k, v, rpe_matrix, trace=False):
    nc = get_program()
    in_maps = prepare_inputs(q, k, v, rpe_matrix)
    res = run_bass_kernel_spmd(nc, in_maps, list(range(N_CORES)), trace=trace)
    dev = np.stack(
        [np.asarray(res.results[c]["o"]) for c in range(N_CORES)]
    )  # [8, 4, 8, 128, 65]
    dev = dev.reshape(NBH, N, 65).astype(np.float64)

    # host-side "+1"-term correction (cumsum of [v, ones]) and the divide
    v64 = np.asarray(v, dtype=np.float64).reshape(NBH, N, D)
    va64 = np.concatenate([v64, np.ones((NBH, N, 1), np.float64)], axis=2)
    pt = np.cumsum(va64, axis=1)  # [32, 1024, 65]
    num = dev[..., :64] + pt[..., :64]
    den = dev[..., 64:65] + pt[..., 64:65]
    o = (num / den).astype(np.float32).reshape(B, H, N, D)
    return o, res


def kernel(q, k, v, drop_noise=None, rpe_matrix=None, p=2, **kw):
    o, _ = run(q, k, v, rpe_matrix)
    return o


if __name__ == "__main__":
    rng = np.random.default_rng(0)
    q = rng.standard_normal((B, H, N, D), dtype=np.float32)
    k = rng.standard_normal((B, H, N, D), dtype=np.float32)
    v = rng.standard_normal((B, H, N, D), dtype=np.float32)
    rpe = rng.standard_normal((2 * N - 1, D), dtype=np.float32)
    o, _ = run(q, k, v, rpe)
    print("out", o.shape, o.dtype, np.abs(o).max())
